# revision 7
# baseline (speedup 1.0000x reference)
"""Trainium2 Bass kernel for ConvFourierKANLayer — Winograd F(2,3) along H.

y = conv2d(cos(x*k), w0) + conv2d(sin(x*k), w1) + bias, k = 1..10,
3x3 kernel, pad 1, C=64 -> O=128, B=16 data-parallel over 8 cores.

vs the direct-conv baseline (90 bf16 matmuls per output strip), the 3
H-taps are Winograd-factored F(2,3): 4 transformed positions produce 2
output rows, cutting PE matmul columns by 1/3. The H-transform is 4
dense bf16 tensor_tensor combos per j (sin+cos merged in one tile);
the inverse (4->2 rows) runs on DVE from PSUM with the bias folded
into fused scalar_tensor_tensor ops.

Activation generation per Fourier order pair j (u, v on ScalarE —
the fp32 fma in ACT's affine stage performs the magic rounding):
  u  = x*(k/2pi)              (ScalarE Copy, per-partition AP scale)
  v  = x*(k/2pi) + 1.5*2^23   (ScalarE Copy; fma rounds to M2+round(u))
  nw = (v - M2) - u = -w      (DVE scalar_tensor_tensor, exact)
  sin(kx) = Sin(-2pi*nw)      (ScalarE spline, valid on [-pi,pi])
  a  = |w|                    (DVE tensor_scalar bitwise_and sign clear)
  cos(kx) = Sin(-2pi*a+pi/2)  (= cos(2pi*|w|) = cos(2pi*w), in-domain)
"""

import numpy as np

import concourse.bass as bass
import concourse.mybir as mybir
import concourse.tile as tile
from concourse import bacc
from concourse.bass_utils import run_bass_kernel_spmd

N_CORES = 8
B, C, H, W = 16, 64, 64, 64
O = 128
G = 10
BS = B // N_CORES  # batches per core
NTW = 120  # weight tiles: j(5) x br(2) x dw(3) x i(4)

PI = float(np.pi)
TWO_PI = float(2 * np.pi)
M2 = 12582912.0  # 1.5*2^23: fp32 round-to-nearest magic (ulp=1 band)

F32 = mybir.dt.float32
U32 = mybir.dt.uint32

_CACHE = {}


def _build_module(reps=1, mmdt="bf16"):
    MMDT = {"bf16": mybir.dt.bfloat16, "fp16": mybir.dt.float16}[mmdt]
    nc = bacc.Bacc("TRN2", target_bir_lowering=False)
    x_d = nc.dram_tensor("x", [BS, C, H, W], F32, kind="ExternalInput")
    w_d = nc.dram_tensor("w", [128, NTW, 128], MMDT, kind="ExternalInput")
    kv_d = nc.dram_tensor("kvec", [128, 5], F32, kind="ExternalInput")
    bias_d = nc.dram_tensor("biasv", [128, 1], F32, kind="ExternalInput")
    y_d = nc.dram_tensor("y", [BS, O, H, W], F32, kind="ExternalOutput")

    add = mybir.AluOpType.add
    subtract = mybir.AluOpType.subtract
    bitwise_and = mybir.AluOpType.bitwise_and
    sin_f = mybir.ActivationFunctionType.Sin
    copy_f = mybir.ActivationFunctionType.Copy
    ident_f = mybir.ActivationFunctionType.Identity

    with tile.TileContext(nc) as tc:
        with (
            tc.tile_pool(name="const", bufs=1) as cpool,
            tc.tile_pool(name="wpool", bufs=1) as wpool,
            tc.tile_pool(name="gen", bufs=2) as gen,
            tc.tile_pool(name="cspool", bufs=2) as cspool,
            tc.tile_pool(name="tpool", bufs=3) as tpool,
            tc.tile_pool(name="invp", bufs=2) as invp,
            tc.tile_pool(name="outp", bufs=2) as outp,
            tc.tile_pool(name="psum", bufs=1, space="PSUM") as psum,
        ):
            wt = wpool.tile([128, NTW, 128], MMDT)
            for j in range(5):
                # j-major consumption order on the Act HWDGE queue
                nc.scalar.dma_start(
                    wt[:, 24 * j : 24 * (j + 1), :],
                    w_d[:, 24 * j : 24 * (j + 1), :],
                )
            kvt = cpool.tile([128, 5], F32)
            nc.sync.dma_start(kvt[:], kv_d[:])
            bt = cpool.tile([128, 1], F32)
            nc.sync.dma_start(bt[:], bias_d[:])
            pih = cpool.tile([128, 1], F32)
            nc.gpsimd.memset(pih[:], PI / 2)

            for rep in range(reps):
              for b in range(BS):
                for blk in range(2):
                    # local rows 0..33 = padded rows 32*blk .. 32*blk+33
                    if blk == 0:
                        rs = slice(1, 34)   # x rows 0..32
                        gr = (0, 33)
                        zrow = 0
                    else:
                        rs = slice(0, 33)   # x rows 31..63
                        gr = (31, 64)
                        zrow = 33

                    xd = gen.tile([128, 34, W], F32, tag="xd")
                    nc.sync.dma_start(
                        xd[0:64, rs, :], x_d[b, :, gr[0] : gr[1], :]
                    )
                    nc.gpsimd.dma_start(
                        xd[64:128, rs, :], x_d[b, :, gr[0] : gr[1], :]
                    )

                    pss = [
                        psum.tile([128, 8, 64], F32, tag=f"ps{mc}{i}",
                                  name=f"ps{mc}{i}_{rep}_{b}_{blk}")
                        for mc in range(2) for i in range(4)
                    ]

                    for j in range(5):
                        ut = gen.tile([128, 34, W], F32, tag="ut")
                        vt = gen.tile([128, 34, W], F32, tag="vt")
                        nwt = gen.tile([128, 34, W], F32, tag="nwt")
                        at = vt  # |w| overwrites v (dead after nw)
                        # sin plane 0, cos plane 1, shared borders
                        sc = cspool.tile([128, 2, 34, W + 2], MMDT,
                                         tag=f"sc{blk}", name="sc")

                        # u = x*(k/2pi); v = fl(x*(k/2pi) + M2) = M2+round(u)
                        nc.scalar.activation(
                            ut[:, rs, :], xd[:, rs, :], copy_f,
                            bias=0.0, scale=kvt[:, j : j + 1],
                        )
                        nc.scalar.activation(
                            vt[:, rs, :], xd[:, rs, :], copy_f,
                            bias=M2, scale=kvt[:, j : j + 1],
                        )
                        # nw = (v - M2) - u = round(u) - u = -w   (exact)
                        nc.vector.scalar_tensor_tensor(
                            nwt[:, rs, :], vt[:, rs, :], -M2, ut[:, rs, :],
                            add, subtract,
                        )
                        # |w| for cos: cos(2pi w) = sin(pi/2 - 2pi|w|)
                        nc.vector.tensor_scalar(
                            at[:, rs, :].bitcast(U32),
                            nwt[:, rs, :].bitcast(U32),
                            0x7FFFFFFF, None, bitwise_and,
                        )
                        nc.scalar.activation(
                            sc[:, 0, rs, 1 : W + 1], nwt[:, rs, :], sin_f,
                            scale=-TWO_PI,
                        )
                        nc.scalar.activation(
                            sc[:, 1, rs, 1 : W + 1], at[:, rs, :], sin_f,
                            bias=pih[:, 0:1], scale=-TWO_PI,
                        )
                        if rep == 0 and b == 0 and j < 2:
                            # zero conv borders; persists across pool reuse
                            # (tag per blk; nothing else writes these bytes)
                            nc.gpsimd.memset(sc[:, :, :, 0:1], 0)
                            nc.gpsimd.memset(sc[:, :, :, W + 1 : W + 2], 0)
                            nc.gpsimd.memset(sc[:, :, zrow : zrow + 1, :], 0)

                        # F(2,3) H-transform, sin+cos in one op via 4D APs
                        e0 = sc
                        tts = [
                            tpool.tile([128, 2, 16, W + 2], MMDT,
                                       tag=f"t{i}", name=f"t{i}")
                            for i in range(4)
                        ]
                        nc.vector.tensor_sub(
                            tts[0][:], e0[:, :, 0:32:2, :], e0[:, :, 2:34:2, :]
                        )
                        nc.vector.tensor_add(
                            tts[1][:], e0[:, :, 1:33:2, :], e0[:, :, 2:34:2, :]
                        )
                        nc.vector.tensor_sub(
                            tts[2][:], e0[:, :, 2:34:2, :], e0[:, :, 1:33:2, :]
                        )
                        nc.vector.tensor_sub(
                            tts[3][:], e0[:, :, 1:33:2, :], e0[:, :, 3:34:2, :]
                        )

                        for br in range(2):
                            for dw in range(3):
                                for i in (1, 2, 0, 3):
                                    for mc in range(2):
                                        # mc innermost: one LDWEIGHTS per 2 MMs
                                        nc.tensor.matmul(
                                            pss[4 * mc + i][:],
                                            wt[:, ((j * 2 + br) * 3 + dw) * 4 + i, :],
                                            tts[i][
                                                :, br,
                                                8 * mc : 8 * mc + 8,
                                                dw : dw + 64,
                                            ],
                                            start=(j == 0 and br == 0
                                                   and dw == 0),
                                            stop=(j == 4 and br == 1
                                                  and dw == 2),
                                        )

                    # inverse: y_even = M0+M1+M2+b, y_odd = M1-M2-M3+b
                    for mc in range(2):
                        M0, M1, M2_ = (pss[4 * mc + i] for i in range(3))
                        M3 = pss[4 * mc + 3]
                        m1 = invp.tile([128, 8, 64], F32, tag="m1")
                        c2 = invp.tile([128, 8, 64], F32, tag="c2")
                        te = invp.tile([128, 8, 64], F32, tag="te")
                        to = invp.tile([128, 8, 64], F32, tag="to")
                        yt = outp.tile([128, 16, 64], F32, tag="yt")
                        nc.scalar.activation(m1[:], M1[:], ident_f)
                        nc.scalar.activation(c2[:], M2_[:], ident_f)
                        # te = (M0 + bias) + m1 ; y_even = te + c2
                        nc.vector.scalar_tensor_tensor(
                            te[:], M0[:], bt[:, 0:1], m1[:], add, add
                        )
                        nc.vector.tensor_add(yt[:, 0:16:2, :], te[:], c2[:])
                        # to = (m1 + bias) - c2 ; y_odd = to - M3
                        nc.vector.scalar_tensor_tensor(
                            to[:], m1[:], bt[:, 0:1], c2[:], add, subtract
                        )
                        nc.vector.tensor_sub(yt[:, 1:16:2, :], to[:], M3[:])
                        eng = nc.gpsimd if mc == 0 else nc.sync
                        h0 = blk * 32 + mc * 16
                        eng.dma_start(y_d[b, :, h0 : h0 + 16, :], yt[:])
    nc.finalize()
    return nc


def _get_module(reps=1, mmdt="bf16"):
    key = ("nc", reps, mmdt)
    if key not in _CACHE:
        _CACHE[key] = _build_module(reps, mmdt)
    return _CACHE[key]


def _np_mmdt(mmdt):
    import ml_dtypes
    return {"bf16": ml_dtypes.bfloat16, "fp16": np.float16}[mmdt]


def _host_weights(fc, mmdt="bf16"):
    # fc: (2, O, C, kH, kW, G); br=0 is the SIN branch (fouriercoeffs[1]).
    # Winograd G-matrix on the kH axis; tile order ((j*2+br)*3+dw)*4+i,
    # tile layout [p=(gp,c), o].
    fc = np.asarray(fc, np.float64)
    Gm = np.array([[1, 0, 0], [0.5, 0.5, 0.5], [0.5, -0.5, 0.5], [0, 0, 1]],
                  np.float64)
    out = np.zeros((128, NTW, 128), np.float64)
    for br in range(2):
        wbr = fc[1 - br]  # (O, C, kh, kw, G)
        gw = np.einsum("ik,ockwg->ociwg", Gm, wbr)  # (O, C, 4, 3, G)
        for j in range(5):
            for dw in range(3):
                for i in range(4):
                    t = ((j * 2 + br) * 3 + dw) * 4 + i
                    for gp in range(2):
                        g = 2 * j + gp
                        out[64 * gp : 64 * (gp + 1), t, :] = (
                            gw[:, :, i, dw, g].T
                        )
    return np.ascontiguousarray(out.astype(_np_mmdt(mmdt)))


def _host_kvec():
    kvec = np.zeros((128, 5), np.float32)
    for j in range(5):
        kvec[0:64, j] = (2 * j + 1) / TWO_PI
        kvec[64:128, j] = (2 * j + 2) / TWO_PI
    return kvec


def kernel(x, fouriercoeffs, bias):
    x = np.ascontiguousarray(np.asarray(x, dtype=np.float32))
    fc = np.asarray(fouriercoeffs, dtype=np.float32)
    w_host = _host_weights(fc)
    kvec = _host_kvec()
    biasv = np.ascontiguousarray(
        np.asarray(bias, dtype=np.float32).reshape(128, 1)
    )

    nc = _get_module()
    in_maps = [
        {"x": x[i * BS : (i + 1) * BS], "w": w_host, "kvec": kvec,
         "biasv": biasv}
        for i in range(N_CORES)
    ]
    res = run_bass_kernel_spmd(nc, in_maps, list(range(N_CORES))).results
    return np.concatenate([res[i]["y"] for i in range(N_CORES)], axis=0)
